# revision 5
# baseline (speedup 1.0000x reference)
"""Trainium2 Bass kernel for BinaryLinear: y = x @ sign(weight).T

Full shapes: x [32, 4096, 1024] f32, weight [1024, 1024] f32 -> y [32, 4096, 1024] f32.
Sharding: data-parallel over tokens across 8 NeuronCores (16384 tokens each).

Host prep (per core): x shard cast to f16 and transposed to xT [1024 i, 16384 t];
weight binarized+transposed+relaid to bH [128, 8k, 1024 o] f16. The device
computes yT [1024 o, 16384 t] f16 = sign(w) @ x.T; the host transposes back and
casts to f32. Pre-transposing on the host removes the on-chip xbar transpose
(the baseline's DMA bottleneck) and lets every DMA be large and contiguous.

Device schedule (per core), tokens in 4 groups of 4096:
  gpsimd+sync:    per-k xT loads [128, 4096] f16, alternating queues
                  (first matmul can start after ~1 MB has landed)
  tensor:         per out-chunk oc: 8 k-chunks x 8 psum banks of [128 o, 512 t],
                  weight-stationary (LDWEIGHTS overlaps the matmul stream),
                  accumulate k in PSUM
  vector/scalar:  PSUM -> SBUF f16 evacuation (one engine per bank of a pair)
  scalar (HWDGE): yT stores [128, 1024] f16 per bank pair
"""

from contextlib import ExitStack

import numpy as np

import concourse.bass as bass
import concourse.mybir as mybir
import concourse.tile as tile
from concourse import bacc
from concourse.bass import ts
from concourse.bass_utils import run_bass_kernel_spmd

P = 128
N_CORES = 8
F32 = mybir.dt.float32
F16 = mybir.dt.float16

FULL_B, FULL_S, D_IN = 32, 4096, 1024
D_OUT = 1024
TOKENS_PER_CORE = FULL_B * FULL_S // N_CORES  # 16384

KC = D_IN // P      # 8 contraction chunks of 128
OC = D_OUT // P     # 8 output chunks of 128
TB = 512            # tokens per psum bank
NB = 8              # psum banks
TSUPER = TB * NB    # 4096 tokens per group
NTG = TOKENS_PER_CORE // TSUPER  # 4 groups


def build_nc(tokens=TOKENS_PER_CORE, d_in=D_IN, d_out=D_OUT):
    """Per-core program: yT[o, t] = sum_i sign(w)[o, i] * x[t, i]."""
    assert tokens % TSUPER == 0
    ntg = tokens // TSUPER

    nc = bacc.Bacc("TRN2")
    xT = nc.dram_tensor("xT", [d_in, tokens], F16, kind="ExternalInput")
    bH = nc.dram_tensor("bH", [P, KC * d_out], F16, kind="ExternalInput")
    yT = nc.dram_tensor("yT", [d_out, tokens], F16, kind="ExternalOutput")

    xT_g = xT.rearrange("(c p) (g t) -> g c p t", p=P, t=TSUPER)
    yT_g = yT.rearrange("(c p) (g t) -> c g p t", p=P, t=TSUPER)
    bH_r = bH.rearrange("p (c o) -> c p o", o=d_out)

    with tile.TileContext(nc) as tc, ExitStack() as ctx:
        bpool = ctx.enter_context(tc.tile_pool(name="b", bufs=KC))
        xpool = ctx.enter_context(tc.tile_pool(name="x", bufs=2 * KC))
        pspool = ctx.enter_context(tc.tile_pool(name="ps", bufs=NB, space="PSUM"))
        opool = ctx.enter_context(tc.tile_pool(name="out", bufs=8))

        # binarized weight, per-k tiles: Bk[k][p, o] = sign(w)[o, k*128+p]
        Bk = []
        for k in range(KC):
            bt = bpool.tile([P, d_out], F16, name="bt")
            nc.scalar.dma_start(bt, bH_r[k])
            Bk.append(bt)

        xtiles = {}

        def load_group(g):
            tiles = []
            for k in range(KC):
                xt = xpool.tile([P, TSUPER], F16, name="xt")
                eng = nc.gpsimd if k % 2 == 0 else nc.sync
                eng.dma_start(xt, xT_g[g, k])
                tiles.append(xt)
            xtiles[g] = tiles

        load_group(0)
        for g in range(ntg):
            if g + 1 < ntg:
                load_group(g + 1)
            xk = xtiles.pop(g)
            for oc in range(OC):
                ps = [pspool.tile([P, TB], F32, name="ps") for _ in range(NB)]
                for k in range(KC):
                    for tb in range(NB):
                        nc.tensor.matmul(
                            ps[tb],
                            Bk[k][:, ts(oc, P)],
                            xk[k][:, ts(tb, TB)],
                            start=(k == 0),
                            stop=(k == KC - 1),
                        )
                for pair in range(NB // 2):
                    out = opool.tile([P, 2 * TB], F16, name="out")
                    nc.vector.tensor_copy(out[:, ts(0, TB)], ps[2 * pair])
                    nc.scalar.copy(out[:, ts(1, TB)], ps[2 * pair + 1])
                    nc.scalar.dma_start(
                        yT_g[oc, g][:, ts(pair, 2 * TB)], out
                    )
    nc.compile()
    return nc


_NC_CACHE = {}


def _get_nc():
    key = (TOKENS_PER_CORE, D_IN, D_OUT)
    if key not in _NC_CACHE:
        _NC_CACHE[key] = build_nc()
    return _NC_CACHE[key]


def run(x, weight, trace=False, **kwargs):
    """Shard, execute on 8 cores, gather. Returns (y_full, BassKernelResults)."""
    x = np.asarray(x)
    weight = np.asarray(weight, dtype=np.float32)
    assert x.shape == (FULL_B, FULL_S, D_IN), x.shape
    assert weight.shape == (D_OUT, D_IN), weight.shape

    x_flat = x.reshape(FULL_B * FULL_S, D_IN)
    # bH[p, k*1024 + o] = sign(w)[o, k*128 + p]
    bH = np.ascontiguousarray(
        np.sign(weight).astype(np.float16).T.reshape(KC, P, D_OUT).transpose(1, 0, 2)
    ).reshape(P, KC * D_OUT)
    in_maps = []
    for c in range(N_CORES):
        shard = x_flat[c * TOKENS_PER_CORE : (c + 1) * TOKENS_PER_CORE]
        xT = np.ascontiguousarray(shard.astype(np.float16).T)
        in_maps.append({"xT": xT, "bH": bH})

    nc = _get_nc()
    res = run_bass_kernel_spmd(
        nc, in_maps, core_ids=list(range(N_CORES)), trace=trace, **kwargs
    )
    y = np.concatenate(
        [res.results[c]["yT"].T for c in range(N_CORES)], axis=0
    ).astype(np.float32)
    return y.reshape(FULL_B, FULL_S, D_OUT), res


def kernel(x, weight):
    try:
        y, _ = run(x, weight)
    except Exception:
        # A freshly-loaded NEFF occasionally faults on its first execution
        # (device-side NRT_EXEC_UNIT_UNRECOVERABLE); one retry has always
        # recovered in testing.
        y, _ = run(x, weight)
    return y


# revision 6
# speedup vs baseline: 1.0697x; 1.0697x over previous
"""Trainium2 Bass kernel for BinaryLinear: y = x @ sign(weight).T

Full shapes: x [32, 4096, 1024] f32, weight [1024, 1024] f32 -> y [32, 4096, 1024] f32.
Sharding: data-parallel over tokens across 8 NeuronCores (16384 tokens each).

Host prep (per core): x shard cast to f16 and transposed to xT [1024 i, 16384 t];
weight binarized+transposed+relaid to bH [128, 8k * 1024 o] f16. The device
computes yT [1024 o, 16384 t] f16 = sign(w) @ x.T; the host transposes back and
casts to f32. Pre-transposing on the host removes the on-chip xbar transpose
(the baseline's DMA bottleneck) and lets every DMA be large and contiguous.

Device schedule (per core): token groups in a staircase (512, 512, 1024, 2048,
4096, 4096, 4096) so the early groups land before the PE can consume them and
the matmul stream starts ~10us in with no stalls; B is split across both load
queues at the head. Per group and out-chunk oc: 8 k-chunks x nb psum banks of
[128 o, 512 t], weight-stationary; LDWEIGHTS overlaps the matmul stream.
PSUM -> SBUF f16 evacuation alternates vector/scalar per bank pair; stores go
out per pair on the scalar HWDGE queue.
"""

from contextlib import ExitStack

import numpy as np

import concourse.bass as bass
import concourse.mybir as mybir
import concourse.tile as tile
from concourse import bacc
from concourse.bass import ts
from concourse.bass_utils import run_bass_kernel_spmd

P = 128
N_CORES = 8
F32 = mybir.dt.float32
F16 = mybir.dt.float16

FULL_B, FULL_S, D_IN = 32, 4096, 1024
D_OUT = 1024
TOKENS_PER_CORE = FULL_B * FULL_S // N_CORES  # 16384

KC = D_IN // P      # 8 contraction chunks of 128
OC = D_OUT // P     # 8 output chunks of 128
TB = 512            # tokens per psum bank
NB = 8              # psum banks
TSUPER = TB * NB    # 4096 tokens max per group

GROUPS = [512, 512, 1024, 2048, 4096, 4096, 4096]
assert sum(GROUPS) == TOKENS_PER_CORE


def build_nc(tokens=TOKENS_PER_CORE, d_in=D_IN, d_out=D_OUT):
    """Per-core program: yT[o, t] = sum_i sign(w)[o, i] * x[t, i]."""
    nc = bacc.Bacc("TRN2")
    xT = nc.dram_tensor("xT", [d_in, tokens], F16, kind="ExternalInput")
    bH = nc.dram_tensor("bH", [P, KC * d_out], F16, kind="ExternalInput")
    yT = nc.dram_tensor("yT", [d_out, tokens], F16, kind="ExternalOutput")

    xT_p = xT.rearrange("(c p) t -> p c t", p=P)
    yT_r = yT.rearrange("(c p) t -> c p t", p=P)

    offs = [0]
    for gsz in GROUPS:
        offs.append(offs[-1] + gsz)

    with tile.TileContext(nc) as tc, ExitStack() as ctx:
        bpool = ctx.enter_context(tc.tile_pool(name="b", bufs=1))
        xpool = ctx.enter_context(tc.tile_pool(name="x", bufs=2))
        pspool = ctx.enter_context(tc.tile_pool(name="ps", bufs=NB, space="PSUM"))
        opool = ctx.enter_context(tc.tile_pool(name="out", bufs=8))

        # binarized weight: B[p, k*1024 + o] = sign(w)[o, k*128 + p]
        B = bpool.tile([P, KC * d_out], F16, name="B")
        half = KC * d_out // 2
        nc.gpsimd.dma_start(B[:, :half], bH[:, :half])
        nc.sync.dma_start(B[:, half:], bH[:, half:])

        xtiles = {}

        def load_group(g):
            xt = xpool.tile([P, KC, TSUPER], F16, name="xt")
            gsz, t0 = GROUPS[g], offs[g]
            eng = nc.gpsimd if g % 2 == 0 else nc.sync
            eng.dma_start(xt[:, :, :gsz], xT_p[:, :, t0 : t0 + gsz])
            xtiles[g] = xt

        load_group(0)
        load_group(1)
        for g in range(len(GROUPS)):
            if g + 2 < len(GROUPS):
                load_group(g + 2)
            gsz, t0 = GROUPS[g], offs[g]
            nb = gsz // TB
            xt = xtiles.pop(g)
            for oc in range(OC):
                ps = [pspool.tile([P, TB], F32, name="ps") for _ in range(nb)]
                for k in range(KC):
                    for tb in range(nb):
                        nc.tensor.matmul(
                            ps[tb],
                            B[:, k * d_out + oc * P : k * d_out + (oc + 1) * P],
                            xt[:, k, ts(tb, TB)],
                            start=(k == 0),
                            stop=(k == KC - 1),
                        )
                for j in range(0, nb, 2):
                    out = opool.tile([P, 2 * TB], F16, name="out")
                    pair_w = min(2, nb - j) * TB
                    nc.vector.tensor_copy(out[:, :TB], ps[j])
                    if nb - j > 1:
                        nc.scalar.copy(out[:, TB : 2 * TB], ps[j + 1])
                    nc.scalar.dma_start(
                        yT_r[oc][:, t0 + j * TB : t0 + j * TB + pair_w],
                        out[:, :pair_w],
                    )
    nc.compile()
    return nc


_NC_CACHE = {}


def _get_nc():
    key = (TOKENS_PER_CORE, D_IN, D_OUT)
    if key not in _NC_CACHE:
        _NC_CACHE[key] = build_nc()
    return _NC_CACHE[key]


def run(x, weight, trace=False, **kwargs):
    """Shard, execute on 8 cores, gather. Returns (y_full, BassKernelResults)."""
    x = np.asarray(x)
    weight = np.asarray(weight, dtype=np.float32)
    assert x.shape == (FULL_B, FULL_S, D_IN), x.shape
    assert weight.shape == (D_OUT, D_IN), weight.shape

    x_flat = x.reshape(FULL_B * FULL_S, D_IN)
    # bH[p, k*1024 + o] = sign(w)[o, k*128 + p]
    bH = np.ascontiguousarray(
        np.sign(weight).astype(np.float16).T.reshape(KC, P, D_OUT).transpose(1, 0, 2)
    ).reshape(P, KC * D_OUT)
    in_maps = []
    for c in range(N_CORES):
        shard = x_flat[c * TOKENS_PER_CORE : (c + 1) * TOKENS_PER_CORE]
        xT = np.ascontiguousarray(shard.astype(np.float16).T)
        in_maps.append({"xT": xT, "bH": bH})

    nc = _get_nc()
    res = run_bass_kernel_spmd(
        nc, in_maps, core_ids=list(range(N_CORES)), trace=trace, **kwargs
    )
    y = np.concatenate(
        [res.results[c]["yT"].T for c in range(N_CORES)], axis=0
    ).astype(np.float32)
    return y.reshape(FULL_B, FULL_S, D_OUT), res


def kernel(x, weight):
    try:
        y, _ = run(x, weight)
    except Exception:
        # A freshly-loaded NEFF occasionally faults on its first execution
        # (device-side NRT_EXEC_UNIT_UNRECOVERABLE); one retry has always
        # recovered in testing.
        y, _ = run(x, weight)
    return y
